# revision 42
# baseline (speedup 1.0000x reference)
"""AxialAttention (MSA row attention) on 8 Trainium2 NeuronCores.

Sharding: data parallel over MSA rows r=128 (16 rows/core); the edge-bias
precompute is sharded over the edge i dim (32 i-rows/core) in kernel 1,
post-processed + gathered on host, replicated into kernel 2.

v5 design notes (on top of the v2 baseline; 363us -> ~251us):
- exp identity: exp(qk + bias) = exp(qk) * exp(bias).  Host ships
  ebt = exp(bias) (edge-masked entries -> exactly 0), the PE bias-inject
  matmuls are gone, and the bias enters as an elementwise multiply on
  DVE/GpSimd after the exp.
- q/k are DMA-shuffled (partition remap) into a [32, ...] layout in
  4-row chunks so every QK matmul runs untiled at partitions 0:32;
  probe1 showed those run back-to-back at full PE speed with no hang,
  while probe2 showed mixing row-tiled and col-tiled matmuls corrupts
  results, so tile_position is only used for the attn@v col bands.
- all DMAs are issued from sync/scalar (HWDGE) only: a dma_start on
  gpsimd blocks the DSP datapath for ~700ns (DIRECT2D), on sync/scalar
  it runs on the sequencer in parallel with the datapath.
- masked-row fixup is a 3-op stt chain: t1 = av*rbig;
  d = (t1 - vbar)*m;  ogr = (d + vbar)*sig  (exact, no copy_predicated).
- LayerNorm sqrt/recip/nmr run batched on [128, 4] tiles and the LN
  apply alternates scalar activation / DVE tensor_scalar.
- every dma_start costs ~620ns of serial sequencer time, so constants
  ship as one packed blob, x as one pre-arranged contiguous tile, and
  the q/k shuffle runs 4 three-dim-AP DMAs per 4-row chunk.
- the HAM power monitor halves the PE clock (2.4 -> 1.2 GHz) after
  ~70us of sustained activity; phase C runs throttled and sits within
  ~10% of that clock's matmul floor.
"""

import sys
import numpy as np
import ml_dtypes

sys.path.insert(0, "/opt/trn_rl_repo")

import concourse.bacc as bacc
import concourse.tile as tile
import concourse.bass as bass
from concourse import mybir
from concourse import bass_utils

F32 = mybir.dt.float32
BF16 = mybir.dt.bfloat16
U8 = mybir.dt.uint8
AF = mybir.ActivationFunctionType
MUL = mybir.AluOpType.mult
ADD = mybir.AluOpType.add
SUB = mybir.AluOpType.subtract

NC = 8          # cores
B, R, W, DN = 1, 128, 256, 256
DE, H, DH = 128, 8, 32
RPC = R // NC   # rows per core = 16
IPC = W // NC   # edge i-rows per core = 32
NEG = -1.0e38
EPS = 1e-5

NB = 2                      # head blocks, 4 heads each (dense)
SLOTS = NB * 128            # 256
P = 128
TOK = RPC * W               # 4096 tokens per core
NT = TOK // P               # 32 token tiles
RCH = 4                     # rows per q/k shuffle chunk
NCH = RPC // RCH            # 4 chunks
# bf16 constant blob layout (columns):
# wq(512) wk(512) wv(512) wg(512) wo(512) bo(256) idm(128) onesb(128)
WB_COLS = 2560 + 256 + 128 + 128  # 3072


def _head_slot(h):
    return (h // 4) * 128 + 32 * (h % 4)


def _expand_cols(Wm):
    D = Wm.shape[0]
    out = np.zeros((D, SLOTS), Wm.dtype)
    for h in range(H):
        out[:, _head_slot(h):_head_slot(h) + DH] = Wm[:, h * DH:(h + 1) * DH]
    return out


def _expand_rows(Wm):
    D = Wm.shape[1]
    out = np.zeros((SLOTS, D), Wm.dtype)
    for h in range(H):
        out[_head_slot(h):_head_slot(h) + DH, :] = Wm[h * DH:(h + 1) * DH, :]
    return out


# ---------------------------------------------------------------- kernel 1
def _build_bias_nc():
    """Per core: pre-transposed edges slice eT [DE, IPC*W] (bf16, host
    transposes for free) -> raw [10, IPC*W]: rows 0:8 = sum_e e*we[e,h];
    row 8 = mean_e e; row 9 = sum_e e^2. LN algebra on host."""
    nc = bacc.Bacc("TRN2", target_bir_lowering=False, debug=False,
                   num_devices=NC)
    TOKE = IPC * W  # 8192
    e_d = nc.dram_tensor("e", [DE, TOKE], BF16, kind="ExternalInput").ap()
    wa_d = nc.dram_tensor("wa", [DE, 9], BF16, kind="ExternalInput").ap()
    o_d = nc.dram_tensor("o", [10, TOKE], F32, kind="ExternalOutput").ap()

    with tile.TileContext(nc) as tc:
        with tc.tile_pool(name="cst", bufs=1) as cst, \
             tc.tile_pool(name="work", bufs=6) as work, \
             tc.tile_pool(name="psr", bufs=6, space="PSUM") as psr:
            wa_sb = cst.tile([DE, 9], BF16)
            nc.sync.dma_start(out=wa_sb, in_=wa_d)
            onesc = cst.tile([P, 1], BF16)
            nc.vector.memset(onesc, 1.0)
            eT_all = cst.tile([P, TOKE], BF16)
            # rows 0:9 = head products + mean; partition 32 = sumsq
            # (engine copies cannot cross partitions; the final DMA does
            # the 32 -> 9 remap)
            o_sb = cst.tile([33, TOKE], F32)
            # 4 chunked loads (each dma_start costs ~620ns of sequencer
            # time, so few big transfers; first chunk lands early).
            QN = TOKE // 4
            for h in range(4):
                (nc.sync if h % 2 == 0 else nc.scalar).dma_start(
                    out=eT_all[:, h * QN:(h + 1) * QN],
                    in_=e_d[:, h * QN:(h + 1) * QN])

            GW = 512  # matmul out must stay within one PSUM bank
            for g in range(TOKE // GW):
                eT = eT_all[:, g * GW:(g + 1) * GW]
                sq = work.tile([P, GW], BF16, tag="sq")
                nc.vector.tensor_tensor(out=sq, in0=eT, in1=eT, op=MUL)
                raw_ps = psr.tile([P, GW], F32, tag="raw")
                nc.tensor.matmul(raw_ps[0:9, :], wa_sb[:], eT,
                                 start=True, stop=True)
                nc.tensor.matmul(raw_ps[32:33, :], onesc[:], sq[:],
                                 start=True, stop=True,
                                 tile_position=(0, 32))
                if g % 2 == 0:
                    nc.scalar.copy(o_sb[0:9, g * GW:(g + 1) * GW],
                                   raw_ps[0:9, :])
                    nc.scalar.copy(o_sb[32:33, g * GW:(g + 1) * GW],
                                   raw_ps[32:33, :])
                else:
                    nc.vector.tensor_copy(out=o_sb[0:9, g * GW:(g + 1) * GW],
                                          in_=raw_ps[0:9, :])
                    nc.vector.tensor_copy(
                        out=o_sb[32:33, g * GW:(g + 1) * GW],
                        in_=raw_ps[32:33, :])
                if g % 4 == 3:
                    c0 = (g - 3) * GW
                    nc.sync.dma_start(out=o_d[0:9, c0:c0 + 4 * GW],
                                      in_=o_sb[0:9, c0:c0 + 4 * GW])
                    nc.scalar.dma_start(out=o_d[9:10, c0:c0 + 4 * GW],
                                        in_=o_sb[32:33, c0:c0 + 4 * GW])
    nc.compile()
    return nc


# ---------------------------------------------------------------- kernel 2
def _build_attn_nc():
    nc = bacc.Bacc("TRN2", target_bir_lowering=False, debug=False,
                   num_devices=NC)

    # x pre-arranged host-side to the SBUF layout [p, token-tile, d] so
    # the load is one contiguous-per-partition DMA (the [TOK, DN] layout
    # needs 4096 512B descriptors and takes ~20us to land)
    x_d = nc.dram_tensor("x", [P, NT, DN], BF16, kind="ExternalInput").ap()
    # all bf16 constants packed into one blob (one DMA issue ~620ns of
    # sequencer time each, so fewer+bigger transfers win):
    # wq0|wq1|wk0|wk1|wv0|wv1|wg0|wg1|wo0|wo1|bo|idm|onesb|mf|ebt
    wb_d = nc.dram_tensor("wb", [P, WB_COLS], BF16,
                          kind="ExternalInput").ap()
    ebt_d = nc.dram_tensor("ebt", [P, NB * 2 * 1024], BF16,
                          kind="ExternalInput").ap()
    mf_d = nc.dram_tensor("mf", [RPC, W], BF16, kind="ExternalInput").ap()
    fb_d = nc.dram_tensor("fb", [P, NB + RPC * 2], F32,
                          kind="ExternalInput").ap()
    o_d = nc.dram_tensor("o", [TOK, DN], BF16, kind="ExternalOutput").ap()

    with tile.TileContext(nc, trace_sim=SIM_TRACE) as tc:
        from contextlib import ExitStack
        with ExitStack() as ctx:
            cst = ctx.enter_context(tc.tile_pool(name="cst", bufs=1))

            # ---------------- persistent tiles (2 blob DMAs)
            wb = cst.tile([P, WB_COLS], BF16, tag="wb", name="wb")
            fbt = cst.tile([P, NB + RPC * 2], F32, tag="fbt", name="fbt")
            nc.scalar.dma_start(out=fbt, in_=fb_d)
            mf_all = cst.tile([P, RPC * W], BF16, tag="mf", name="mf")
            nc.scalar.dma_start(
                out=mf_all,
                in_=bass.AP(tensor=mf_d.tensor, offset=0,
                            ap=[[0, P], [1, RPC * W]]))
            ebt_sb = cst.tile([P, NB * 2 * 1024], BF16, tag="ebtt",
                              name="ebtt")

            def wslc(off, wdt):
                return wb[:, off:off + wdt]

            wq = [wslc(kt * SLOTS, SLOTS) for kt in range(2)]
            wk = [wslc(512 + kt * SLOTS, SLOTS) for kt in range(2)]
            wv = [wslc(1024 + kt * SLOTS, SLOTS) for kt in range(2)]
            wg = [wslc(1536 + kt * SLOTS, SLOTS) for kt in range(2)]
            wo = [wslc(2048 + b * DN, DN) for b in range(NB)]
            bo = wb[0:1, 2560:2560 + DN]
            ident = wslc(2816, P)
            ones_sq = wslc(2944, P)
            ones_blk = ones_sq[:, 0:32]
            one1 = ones_sq[0:1, :]
            onecol = ones_sq[:, 0:1]
            bg = fbt[:, 0:NB]
            ngj = fbt[:, NB:NB + RPC * 2]
            eps_sb = cst.tile([P, 1], F32)
            nc.vector.memset(eps_sb, EPS)

            # q (b0,b1) and k (b0,b1) in one [P, 4, TOK] tile so one
            # 3-dim-AP DMA per head band shuffles all four at once
            qk_sb = cst.tile([P, 4, TOK], BF16, tag="qk", name="qk")
            sig_sb = [cst.tile([P, TOK], BF16, tag=f"sg{b}", name=f"sg{b}")
                      for b in range(NB)]
            v_sb = [cst.tile([P, SLOTS], BF16, tag=f"v{tt}", name=f"v{tt}")
                    for tt in range(NT)]
            vbarW_all = cst.tile([P, 2 * RPC], F32, tag="vbw_all",
                                 name="vbw_all")

            # q/k shuffle chunks: [32, H*RCH*W] per chunk, head h at
            # cols h*RCH*W, row r_loc at + r_loc*W.
            CHW = RCH * W
            qsh = ctx.enter_context(tc.tile_pool(name="qsh", bufs=2))

            # ---------------- phase A: LN + transpose + q/k/v projections
            with tc.tile_pool(name="abp", bufs=1) as abp, \
                 tc.tile_pool(name="lnw", bufs=4) as lnw, \
                 tc.tile_pool(name="tpp", bufs=2, space="PSUM") as tpp, \
                 tc.tile_pool(name="pjp", bufs=3, space="PSUM") as pjp:
                xnT = [abp.tile([P, TOK], BF16, tag=f"xnT{kt}",
                                name=f"xnT{kt}") for kt in range(2)]

                def proj_qkg(ws, b, ch, pp):
                    for kt in range(2):
                        nc.tensor.matmul(
                            pp[:], ws[kt][:, b * P:(b + 1) * P],
                            xnT[kt][:, ch * 512:(ch + 1) * 512],
                            start=(kt == 0), stop=(kt == 1))

                drain_rr = [0]

                def drain(dst, src):
                    # gpsimd cannot access PSUM: scalar/vector only
                    e = drain_rr[0] % 2
                    drain_rr[0] += 1
                    if e == 0:
                        nc.scalar.copy(dst, src)
                    else:
                        nc.vector.tensor_copy(out=dst, in_=src)

                xall = abp.tile([P, NT, DN], BF16, tag="xall",
                                name="xall")
                # x chunk 0 first: LN stats for group 0 gate everything
                nc.sync.dma_start(out=xall[:, 0:8, :], in_=x_d[:, 0:8, :])
                nc.sync.dma_start(out=wb, in_=wb_d)
                nc.sync.dma_start(out=xall[:, 8:NT, :],
                                  in_=x_d[:, 8:NT, :])
                nc.scalar.dma_start(out=ebt_sb, in_=ebt_d)
                for g in range(NT // 4):
                    xb = xall[:, 4 * g:4 * g + 4, :]
                    # batched LN stats: 2x bn_stats over 2 chunks each,
                    # then tiny [128, 4] scalar/DVE ops.
                    st = lnw.tile([P, 4, 6], F32, tag="st")
                    for c in range(4):
                        nc.vector.bn_stats(out=st[:, c, :], in_=xb[:, c, :])
                    mv = lnw.tile([P, 4, 2], F32, tag="mv")
                    for c in range(4):
                        nc.vector.bn_aggr(out=mv[:, c, :], in_=st[:, c, :])
                    sd = lnw.tile([P, 4], F32, tag="sd")
                    nc.scalar.activation(sd, mv[:, :, 1], AF.Sqrt,
                                         bias=eps_sb[:])
                    rstd = lnw.tile([P, 4], F32, tag="rs")
                    nc.vector.reciprocal(rstd, sd)
                    nmr = lnw.tile([P, 4], F32, tag="nm")
                    nc.vector.scalar_tensor_tensor(
                        out=nmr, in0=mv[:, :, 0], scalar=-1.0, in1=rstd,
                        op0=MUL, op1=MUL)
                    xng = lnw.tile([P, 4 * DN], BF16, tag="xn")
                    tp_ps = tpp.tile([P, 1024], BF16, tag="tp",
                                     name=f"tp{g}")
                    for c in range(4):
                        xn = xng[:, c * DN:(c + 1) * DN]
                        if c % 2 == 0:
                            nc.scalar.activation(
                                xn, xb[:, c, :], AF.Identity,
                                bias=nmr[:, c:c + 1], scale=rstd[:, c:c + 1])
                        else:
                            nc.vector.tensor_scalar(
                                out=xn, in0=xb[:, c, :],
                                scalar1=rstd[:, c:c + 1],
                                scalar2=nmr[:, c:c + 1],
                                op0=MUL, op1=ADD)
                        for kt in range(2):
                            nc.tensor.transpose(
                                tp_ps[:, kt * 512 + c * P:
                                      kt * 512 + (c + 1) * P],
                                xn[:, kt * P:(kt + 1) * P], ident[:])
                    for kt in range(2):
                        drain(xnT[kt][:, g * 512:(g + 1) * 512],
                              tp_ps[:, kt * 512:(kt + 1) * 512])

                    # q/k/v projections for this chunk
                    ch = g
                    for b in range(NB):
                        pp = pjp.tile([P, 512], F32, tag="pj")
                        proj_qkg(wq, b, ch, pp)
                        drain(qk_sb[:, b, ch * 512:(ch + 1) * 512], pp)
                        pp = pjp.tile([P, 512], F32, tag="pj")
                        proj_qkg(wk, b, ch, pp)
                        drain(qk_sb[:, 2 + b, ch * 512:(ch + 1) * 512], pp)
                    for tl in range(4):
                        tt = g * 4 + tl
                        pp = pjp.tile([P, SLOTS], F32, tag="vps", bufs=2)
                        for kt in range(2):
                            nc.tensor.matmul(
                                pp[:], xnT[kt][:, tt * P:(tt + 1) * P],
                                wv[kt][:], start=(kt == 0), stop=(kt == 1))
                        drain(v_sb[tt][:], pp)

                    for rloc in range(2):
                        r_ = 2 * g + rloc
                        vb_ps = pjp.tile([P, 2], F32, tag="vb", bufs=1)
                        for b in range(NB):
                            for tl in range(2):
                                nc.tensor.matmul(
                                    vb_ps[:, b:b + 1],
                                    v_sb[2 * r_ + tl][:, b * P:(b + 1) * P],
                                    onecol, start=(tl == 0),
                                    stop=(tl == 1))
                        nc.vector.tensor_scalar(
                            out=vbarW_all[:, 2 * r_:2 * r_ + 2], in0=vb_ps,
                            scalar1=1.0 / W, scalar2=None, op0=MUL)

                # g projection + sigmoid (sigmoid act table)
                for ch in range(TOK // 512):
                    for b in range(NB):
                        pp = pjp.tile([P, 512], F32, tag="pj")
                        proj_qkg(wg, b, ch, pp)
                        nc.scalar.activation(
                            sig_sb[b][:, ch * 512:(ch + 1) * 512], pp,
                            AF.Sigmoid, bias=bg[:, b:b + 1])

            # ---------------- phase C: per-row attention (exp table)
            with tc.tile_pool(name="etp", bufs=6) as etp, \
                 tc.tile_pool(name="emp", bufs=6) as emp, \
                 tc.tile_pool(name="rowp", bufs=3) as rowp, \
                 tc.tile_pool(name="ogp", bufs=4) as ogp, \
                 tc.tile_pool(name="dtp", bufs=2, space="PSUM") as dtp, \
                 tc.tile_pool(name="adp", bufs=2, space="PSUM") as adp, \
                 tc.tile_pool(name="opp", bufs=1, space="PSUM") as opp:
                def load_qk_chunk(c):
                    qkt = qsh.tile([32, 4, 4, CHW], BF16, tag="qsh",
                                   name=f"qkc{c}")
                    for u in range(4):
                        (nc.sync if u % 2 == 0 else nc.scalar).dma_start(
                            out=qkt[:, u, :, :],
                            in_=qk_sb[32 * u:32 * u + 32, :,
                                      c * CHW:(c + 1) * CHW])
                    return qkt

                def outproj(pr, pogrs):
                    pt0 = pr * W
                    ot2 = rowp.tile([P, 2 * DN], BF16, tag="ot2")
                    for it in range(2):
                        op_ps = opp.tile([P, DN], F32, tag="op")
                        nc.tensor.matmul(op_ps[:], one1, bo[:],
                                         start=True, stop=False)
                        for b in range(NB):
                            nc.tensor.matmul(
                                op_ps[:], pogrs[b][:, it * P:(it + 1) * P],
                                wo[b][:], start=False, stop=(b == NB - 1))
                        if it % 2 == 0:
                            nc.scalar.copy(ot2[:, 0:DN], op_ps)
                        else:
                            nc.vector.tensor_copy(out=ot2[:, DN:2 * DN],
                                                  in_=op_ps)
                    (nc.sync if pr % 2 == 0 else nc.scalar).dma_start(
                        out=bass.AP(tensor=o_d.tensor, offset=pt0 * DN,
                                    ap=[[DN, P], [P * DN, 2], [1, DN]]),
                        in_=ot2)

                prev = None
                chunks = [load_qk_chunk(0), load_qk_chunk(1)]
                for r in range(DO_C_ROWS):
                    t0 = r * W
                    c, rl = r // RCH, r % RCH
                    qkt = chunks[c % 2]
                    if rl == 0 and c + 1 < NCH and r > 0:
                        chunks[(c + 1) % 2] = load_qk_chunk(c + 1)

                    # 1) dots (untiled, partitions 0:32) + exp + ebias
                    # multiply per (b, jt) group.
                    ems = [[None, None] for _ in range(NB)]
                    for b in range(NB):
                        for jt in range(2):
                            dt_ps = dtp.tile([P, 1024], F32, tag="dt")
                            for u in range(4):
                                hb = rl * W
                                nc.tensor.matmul(
                                    dt_ps[:, u * W:(u + 1) * W],
                                    qkt[:, u, 2 + b,
                                        hb + jt * P:hb + (jt + 1) * P],
                                    qkt[:, u, b, hb:hb + W],
                                    start=True, stop=True)
                            et = etp.tile([P, 1024], BF16, tag="et")
                            nc.scalar.activation(
                                et, dt_ps, AF.Exp,
                                bias=ngj[:, r * 2 + jt:r * 2 + jt + 1])
                            em = emp.tile([P, 1024], BF16, tag="em")
                            eo = (2 * b + jt) * 1024
                            if jt == 0:
                                nc.vector.tensor_tensor(
                                    out=em, in0=et,
                                    in1=ebt_sb[:, eo:eo + 1024], op=MUL)
                            else:
                                # gpsimd is ~3x slower per element: give it
                                # only the last quarter
                                nc.vector.tensor_tensor(
                                    out=em[:, 0:768], in0=et[:, 0:768],
                                    in1=ebt_sb[:, eo:eo + 768], op=MUL)
                                nc.gpsimd.tensor_tensor(
                                    out=em[:, 768:1024],
                                    in0=et[:, 768:1024],
                                    in1=ebt_sb[:, eo + 768:eo + 1024],
                                    op=MUL)
                            ems[b][jt] = em

                    # software pipeline: the previous row's output
                    # projection lands here on the PE queue.
                    if prev is not None:
                        outproj(*prev)
                        prev = None

                    # 2) attn@v + denominators (col-tiled bands; each
                    # accumulation group start->stop contiguous).
                    avdns = []
                    for b in range(NB):
                        avdn = adp.tile([P, 512], F32, tag="ad",
                                        name=f"ad{r}_{b}")
                        ems_b = ems[b]
                        for u in range(4):
                            ho = 32 * u
                            for jt in range(2):
                                nc.tensor.matmul(
                                    avdn[ho:ho + 32, 0:W],
                                    v_sb[2 * r + jt][:, b * P + ho:
                                                     b * P + ho + DH],
                                    ems_b[jt][:, u * W:(u + 1) * W],
                                    start=(jt == 0), stop=(jt == 1),
                                    tile_position=(0, ho))
                            for jt in range(2):
                                nc.tensor.matmul(
                                    avdn[ho:ho + 32, W:2 * W],
                                    ones_blk,
                                    ems_b[jt][:, u * W:(u + 1) * W],
                                    start=(jt == 0), stop=(jt == 1),
                                    tile_position=(0, ho))
                        avdns.append(avdn)

                    # 3) normalize + gate + masked-row fixup:
                    #    ogr = ((av*rbig - vbar)*m + vbar) * sig
                    ogrs = []
                    for b in range(NB):
                        avdn = avdns[b]
                        vcol = vbarW_all[:, 2 * r + b:2 * r + b + 1]
                        rbig = rowp.tile([P, W], F32, tag="rbig")
                        nc.vector.reciprocal_approx_fast(rbig, avdn[:, W:])
                        t1 = rowp.tile([P, W], F32, tag="t1")
                        nc.vector.tensor_tensor(
                            out=t1, in0=avdn[:, 0:W], in1=rbig, op=MUL)
                        d = rowp.tile([P, W], F32, tag="d")
                        nc.vector.scalar_tensor_tensor(
                            out=d, in0=t1, scalar=vcol,
                            in1=mf_all[:, t0:t0 + W], op0=SUB, op1=MUL)
                        ogr = ogp.tile([P, W], BF16, tag=f"og{b}")
                        nc.vector.scalar_tensor_tensor(
                            out=ogr, in0=d, scalar=vcol,
                            in1=sig_sb[b][:, t0:t0 + W], op0=ADD, op1=MUL)
                        ogrs.append(ogr)

                    prev = (r, ogrs)
                if prev is not None:
                    outproj(*prev)
    nc.compile()
    return nc


_NC_CACHE = {}
TRACE = False
SIM_TRACE = False
DO_C_ROWS = RPC
DBG = False


def _get_nc(name):
    if name not in _NC_CACHE:
        _NC_CACHE[name] = (_build_bias_nc if name == "bias"
                           else _build_attn_nc)()
    return _NC_CACHE[name]


def _prep(x, edges, mask, edge_mask, ln_g, ln_b, lne_g, lne_b,
          W_edge, Wq, Wkv, Wg, bg, Wo, bo):
    f32 = np.float32
    bf16 = ml_dtypes.bfloat16
    x = np.asarray(x, f32)
    edges = np.asarray(edges, f32)
    mask_b = np.asarray(mask).astype(bool)
    edge_mask_b = np.asarray(edge_mask).astype(bool)
    ln_g = np.asarray(ln_g, f32); ln_b = np.asarray(ln_b, f32)
    lne_g = np.asarray(lne_g, f32); lne_b = np.asarray(lne_b, f32)
    W_edge = np.asarray(W_edge, f32)
    Wq = np.asarray(Wq, f32); Wkv = np.asarray(Wkv, f32)
    Wg = np.asarray(Wg, f32); bg = np.asarray(bg, f32)
    Wo = np.asarray(Wo, f32); bo = np.asarray(bo, f32)

    # ---------------- kernel 1: raw edge products
    nc1 = _get_nc("bias")
    we = (lne_g[:, None] * W_edge).astype(f32)
    we_bf = we.astype(bf16)
    wa = np.zeros((DE, 9), f32)
    wa[:, 0:8] = we_bf.astype(f32)
    wa[:, 8] = 1.0 / DE
    e_flat = edges.reshape(W, W, DE)
    in_maps1 = []
    for c in range(NC):
        in_maps1.append({
            "e": np.ascontiguousarray(
                e_flat[c * IPC:(c + 1) * IPC].reshape(IPC * W, DE).T
            ).astype(bf16),
            "wa": wa.astype(bf16),
        })
    res1 = _run_spmd(nc1, in_maps1)
    if TRACE:
        print("bias kernel exec_time_ns:", res1.exec_time_ns)
    o1 = np.concatenate([res1.results[c]["o"] for c in range(NC)],
                        axis=1)  # [10, W*W]
    raw = o1[0:8]                        # [8, i*j]
    mu = o1[8]                           # [i*j]
    var = o1[9] / DE - mu * mu
    rstd = 1.0 / np.sqrt(var + EPS)
    swe = we_bf.astype(f32).sum(axis=0)  # [H]
    bias = rstd[None, :] * (raw - mu[None, :] * swe[:, None])
    bias = bias.reshape(H, W, W) + (lne_b @ W_edge)[:, None, None]
    # exp(bias) with masked edges -> exactly 0 (exp identity moves the
    # bias out of the PE: exp(qk+bias) = exp(qk) * exp(bias))
    ebias = np.where(edge_mask_b[0][None], np.exp(bias), 0.0)  # [H, i, j]
    ebT = ebias.transpose(0, 2, 1)                             # [H, j, i]
    ebt = np.ascontiguousarray(
        ebT.reshape(NB, 4, 2, P, W)            # (b, u, jt, p, i)
        .transpose(3, 0, 2, 1, 4)              # (p, b, jt, u, i)
        .reshape(P, NB * 2 * 1024)).astype(f32)

    # ---------------- kernel 2: attention
    nc2 = _get_nc("attn")
    scale = DH ** -0.5
    Wk_, Wv_ = Wkv[:, :H * DH], Wkv[:, H * DH:]
    gq = _expand_cols((ln_g[:, None] * Wq * scale).astype(f32))
    gk = _expand_cols((ln_g[:, None] * Wk_).astype(f32))
    gv = _expand_cols((ln_g[:, None] * Wv_).astype(f32))
    gg = _expand_cols((ln_g[:, None] * Wg).astype(f32))
    assert np.allclose(ln_b, 0.0), "ln_b folding not implemented"
    bgx = np.zeros((P, NB), f32)
    for h in range(H):
        bgx[32 * (h % 4):32 * (h % 4) + DH, h // 4] = \
            bg[h * DH:(h + 1) * DH]
    woe = _expand_rows(Wo.astype(f32))

    maskf = mask_b[0].astype(f32)  # [R, W]
    x_flat = x.reshape(R, W, DN)
    in_maps2 = []
    for c in range(NC):
        mrows = maskf[c * RPC:(c + 1) * RPC]  # [RPC, W]
        ngj = (mrows.reshape(RPC, 2, P) - 1.0) * 1e38  # [r, jt, p]
        ngj = np.ascontiguousarray(
            ngj.transpose(2, 0, 1).reshape(P, RPC * 2))
        # bf16 constant blob: wq|wk|wv|wg|wo|bo(row0)|idm|onesb|mf|ebt
        wbb = np.zeros((P, WB_COLS), f32)
        wbb[:, 0:512] = gq.reshape(2, P, SLOTS).transpose(1, 0, 2) \
            .reshape(P, 512)
        wbb[:, 512:1024] = gk.reshape(2, P, SLOTS) \
            .transpose(1, 0, 2).reshape(P, 512)
        wbb[:, 1024:1536] = gv.reshape(2, P, SLOTS) \
            .transpose(1, 0, 2).reshape(P, 512)
        wbb[:, 1536:2048] = gg.reshape(2, P, SLOTS) \
            .transpose(1, 0, 2).reshape(P, 512)
        wbb[:, 2048:2560] = woe.reshape(2, P, DN).transpose(1, 0, 2) \
            .reshape(P, 512)
        wbb[0, 2560:2816] = bo
        wbb[:, 2816:2944] = np.eye(P, dtype=f32)
        wbb[:, 2944:3072] = 1.0
        fbb = np.concatenate([bgx, ngj], axis=1)
        xc = x_flat[c * RPC:(c + 1) * RPC].reshape(NT, P, DN)
        in_maps2.append({
            "x": np.ascontiguousarray(xc.transpose(1, 0, 2)).astype(bf16),
            "wb": wbb.astype(bf16),
            "fb": fbb.astype(f32),
            "ebt": ebt.astype(bf16),
            "mf": mrows.astype(bf16),
        })
    return nc2, in_maps2


def _run_spmd(nc, in_maps):
    # the axon-tunneled devices occasionally fail one execution with
    # NRT_EXEC_UNIT_UNRECOVERABLE; a retry succeeds
    try:
        return bass_utils.run_bass_kernel_spmd(nc, in_maps,
                                               core_ids=list(range(NC)),
                                               trace=TRACE)
    except Exception:
        return bass_utils.run_bass_kernel_spmd(nc, in_maps,
                                               core_ids=list(range(NC)),
                                               trace=TRACE)


def kernel(**inputs):
    nc2, in_maps2 = _prep(**inputs)
    res2 = _run_spmd(nc2, in_maps2)
    if TRACE:
        print("attn kernel exec_time_ns:", res2.exec_time_ns)
    out = np.concatenate(
        [res2.results[c]["o"].astype(np.float32).reshape(RPC, W, DN)
         for c in range(NC)],
        axis=0)
    return out.reshape(B, R, W, DN).astype(np.float32)


# revision 43
# speedup vs baseline: 1.0053x; 1.0053x over previous
"""AxialAttention (MSA row attention) on 8 Trainium2 NeuronCores.

Sharding: data parallel over MSA rows r=128 (16 rows/core); the edge-bias
precompute is sharded over the edge i dim (32 i-rows/core) in kernel 1,
post-processed + gathered on host, replicated into kernel 2.

v5 design notes (on top of the v2 baseline; 363us -> ~251us):
- exp identity: exp(qk + bias) = exp(qk) * exp(bias).  Host ships
  ebt = exp(bias) (edge-masked entries -> exactly 0), the PE bias-inject
  matmuls are gone, and the bias enters as an elementwise multiply on
  DVE/GpSimd after the exp.
- q/k are DMA-shuffled (partition remap) into a [32, ...] layout in
  4-row chunks so every QK matmul runs untiled at partitions 0:32;
  probe1 showed those run back-to-back at full PE speed with no hang,
  while probe2 showed mixing row-tiled and col-tiled matmuls corrupts
  results, so tile_position is only used for the attn@v col bands.
- all DMAs are issued from sync/scalar (HWDGE) only: a dma_start on
  gpsimd blocks the DSP datapath for ~700ns (DIRECT2D), on sync/scalar
  it runs on the sequencer in parallel with the datapath.
- masked-row fixup is a 3-op stt chain: t1 = av*rbig;
  d = (t1 - vbar)*m;  ogr = (d + vbar)*sig  (exact, no copy_predicated).
- LayerNorm sqrt/recip/nmr run batched on [128, 4] tiles and the LN
  apply alternates scalar activation / DVE tensor_scalar.
- every dma_start costs ~620ns of serial sequencer time, so constants
  ship as one packed blob, x as one pre-arranged contiguous tile, and
  the q/k shuffle runs 4 three-dim-AP DMAs per 4-row chunk.
- the HAM power monitor halves the PE clock (2.4 -> 1.2 GHz) after
  ~70us of sustained activity; phase C runs throttled and sits within
  ~10% of that clock's matmul floor.
"""

import sys
import numpy as np
import ml_dtypes

sys.path.insert(0, "/opt/trn_rl_repo")

import concourse.bacc as bacc
import concourse.tile as tile
import concourse.bass as bass
from concourse import mybir
from concourse import bass_utils

F32 = mybir.dt.float32
BF16 = mybir.dt.bfloat16
U8 = mybir.dt.uint8
AF = mybir.ActivationFunctionType
MUL = mybir.AluOpType.mult
ADD = mybir.AluOpType.add
SUB = mybir.AluOpType.subtract

NC = 8          # cores
B, R, W, DN = 1, 128, 256, 256
DE, H, DH = 128, 8, 32
RPC = R // NC   # rows per core = 16
IPC = W // NC   # edge i-rows per core = 32
NEG = -1.0e38
EPS = 1e-5

NB = 2                      # head blocks, 4 heads each (dense)
SLOTS = NB * 128            # 256
P = 128
TOK = RPC * W               # 4096 tokens per core
NT = TOK // P               # 32 token tiles
RCH = 4                     # rows per q/k shuffle chunk
NCH = RPC // RCH            # 4 chunks
# bf16 constant blob layout (columns):
# wq(512) wk(512) wv(512) wg(512) wo(512) bo(256) idm(128) onesb(128)
WB_COLS = 2560 + 256 + 128 + 128  # 3072


def _head_slot(h):
    return (h // 4) * 128 + 32 * (h % 4)


def _expand_cols(Wm):
    D = Wm.shape[0]
    out = np.zeros((D, SLOTS), Wm.dtype)
    for h in range(H):
        out[:, _head_slot(h):_head_slot(h) + DH] = Wm[:, h * DH:(h + 1) * DH]
    return out


def _expand_rows(Wm):
    D = Wm.shape[1]
    out = np.zeros((SLOTS, D), Wm.dtype)
    for h in range(H):
        out[_head_slot(h):_head_slot(h) + DH, :] = Wm[h * DH:(h + 1) * DH, :]
    return out


# ---------------------------------------------------------------- kernel 1
def _build_bias_nc():
    """Per core: pre-transposed edges slice eT [DE, IPC*W] (bf16, host
    transposes for free) -> raw [10, IPC*W]: rows 0:8 = sum_e e*we[e,h];
    row 8 = mean_e e; row 9 = sum_e e^2. LN algebra on host."""
    nc = bacc.Bacc("TRN2", target_bir_lowering=False, debug=False,
                   num_devices=NC)
    TOKE = IPC * W  # 8192
    e_d = nc.dram_tensor("e", [DE, TOKE], BF16, kind="ExternalInput").ap()
    wa_d = nc.dram_tensor("wa", [DE, 9], BF16, kind="ExternalInput").ap()
    o_d = nc.dram_tensor("o", [10, TOKE], F32, kind="ExternalOutput").ap()

    with tile.TileContext(nc) as tc:
        with tc.tile_pool(name="cst", bufs=1) as cst, \
             tc.tile_pool(name="work", bufs=6) as work, \
             tc.tile_pool(name="psr", bufs=6, space="PSUM") as psr:
            wa_sb = cst.tile([DE, 9], BF16)
            nc.sync.dma_start(out=wa_sb, in_=wa_d)
            onesc = cst.tile([P, 1], BF16)
            nc.vector.memset(onesc, 1.0)
            eT_all = cst.tile([P, TOKE], BF16)
            # rows 0:9 = head products + mean; partition 32 = sumsq
            # (engine copies cannot cross partitions; the final DMA does
            # the 32 -> 9 remap)
            o_sb = cst.tile([33, TOKE], F32)
            # 4 chunked loads (each dma_start costs ~620ns of sequencer
            # time, so few big transfers; first chunk lands early).
            QN = TOKE // 4
            for h in range(4):
                (nc.sync if h % 2 == 0 else nc.scalar).dma_start(
                    out=eT_all[:, h * QN:(h + 1) * QN],
                    in_=e_d[:, h * QN:(h + 1) * QN])

            GW = 512  # matmul out must stay within one PSUM bank
            for g in range(TOKE // GW):
                eT = eT_all[:, g * GW:(g + 1) * GW]
                sq = work.tile([P, GW], BF16, tag="sq")
                nc.vector.tensor_tensor(out=sq, in0=eT, in1=eT, op=MUL)
                raw_ps = psr.tile([P, GW], F32, tag="raw")
                nc.tensor.matmul(raw_ps[0:9, :], wa_sb[:], eT,
                                 start=True, stop=True)
                nc.tensor.matmul(raw_ps[32:33, :], onesc[:], sq[:],
                                 start=True, stop=True,
                                 tile_position=(0, 32))
                if g % 2 == 0:
                    nc.scalar.copy(o_sb[0:9, g * GW:(g + 1) * GW],
                                   raw_ps[0:9, :])
                    nc.scalar.copy(o_sb[32:33, g * GW:(g + 1) * GW],
                                   raw_ps[32:33, :])
                else:
                    nc.vector.tensor_copy(out=o_sb[0:9, g * GW:(g + 1) * GW],
                                          in_=raw_ps[0:9, :])
                    nc.vector.tensor_copy(
                        out=o_sb[32:33, g * GW:(g + 1) * GW],
                        in_=raw_ps[32:33, :])
                if g % 4 == 3:
                    c0 = (g - 3) * GW
                    nc.sync.dma_start(out=o_d[0:9, c0:c0 + 4 * GW],
                                      in_=o_sb[0:9, c0:c0 + 4 * GW])
                    nc.scalar.dma_start(out=o_d[9:10, c0:c0 + 4 * GW],
                                        in_=o_sb[32:33, c0:c0 + 4 * GW])
    nc.compile()
    return nc


# ---------------------------------------------------------------- kernel 2
def _build_attn_nc():
    nc = bacc.Bacc("TRN2", target_bir_lowering=False, debug=False,
                   num_devices=NC)

    # x pre-arranged host-side to the SBUF layout [p, token-tile, d] so
    # the load is one contiguous-per-partition DMA (the [TOK, DN] layout
    # needs 4096 512B descriptors and takes ~20us to land)
    x_d = nc.dram_tensor("x", [P, NT, DN], BF16, kind="ExternalInput").ap()
    # all bf16 constants packed into one blob (one DMA issue ~620ns of
    # sequencer time each, so fewer+bigger transfers win):
    # wq0|wq1|wk0|wk1|wv0|wv1|wg0|wg1|wo0|wo1|bo|idm|onesb|mf|ebt
    wb_d = nc.dram_tensor("wb", [P, WB_COLS], BF16,
                          kind="ExternalInput").ap()
    ebt_d = nc.dram_tensor("ebt", [P, NB * 2 * 1024], BF16,
                          kind="ExternalInput").ap()
    mf_d = nc.dram_tensor("mf", [RPC, W], BF16, kind="ExternalInput").ap()
    fb_d = nc.dram_tensor("fb", [P, NB + RPC * 2], F32,
                          kind="ExternalInput").ap()
    o_d = nc.dram_tensor("o", [TOK, DN], BF16, kind="ExternalOutput").ap()

    with tile.TileContext(nc, trace_sim=SIM_TRACE) as tc:
        from contextlib import ExitStack
        with ExitStack() as ctx:
            cst = ctx.enter_context(tc.tile_pool(name="cst", bufs=1))

            # ---------------- persistent tiles (2 blob DMAs)
            wb = cst.tile([P, WB_COLS], BF16, tag="wb", name="wb")
            fbt = cst.tile([P, NB + RPC * 2], F32, tag="fbt", name="fbt")
            nc.scalar.dma_start(out=fbt, in_=fb_d)
            mf_all = cst.tile([P, RPC * W], BF16, tag="mf", name="mf")
            nc.scalar.dma_start(
                out=mf_all,
                in_=bass.AP(tensor=mf_d.tensor, offset=0,
                            ap=[[0, P], [1, RPC * W]]))
            ebt_sb = cst.tile([P, NB * 2 * 1024], BF16, tag="ebtt",
                              name="ebtt")

            def wslc(off, wdt):
                return wb[:, off:off + wdt]

            wq = [wslc(kt * SLOTS, SLOTS) for kt in range(2)]
            wk = [wslc(512 + kt * SLOTS, SLOTS) for kt in range(2)]
            wv = [wslc(1024 + kt * SLOTS, SLOTS) for kt in range(2)]
            wg = [wslc(1536 + kt * SLOTS, SLOTS) for kt in range(2)]
            wo = [wslc(2048 + b * DN, DN) for b in range(NB)]
            bo = wb[0:1, 2560:2560 + DN]
            ident = wslc(2816, P)
            ones_sq = wslc(2944, P)
            ones_blk = ones_sq[:, 0:32]
            one1 = ones_sq[0:1, :]
            onecol = ones_sq[:, 0:1]
            bg = fbt[:, 0:NB]
            ngj = fbt[:, NB:NB + RPC * 2]
            eps_sb = cst.tile([P, 1], F32)
            nc.vector.memset(eps_sb, EPS)

            # q (b0,b1) and k (b0,b1) in one [P, 4, TOK] tile so one
            # 3-dim-AP DMA per head band shuffles all four at once
            qk_sb = cst.tile([P, 4, TOK], BF16, tag="qk", name="qk")
            sig_sb = [cst.tile([P, TOK], BF16, tag=f"sg{b}", name=f"sg{b}")
                      for b in range(NB)]
            v_sb = [cst.tile([P, SLOTS], BF16, tag=f"v{tt}", name=f"v{tt}")
                    for tt in range(NT)]
            vbarW_all = cst.tile([P, 2 * RPC], F32, tag="vbw_all",
                                 name="vbw_all")

            # q/k shuffle chunks: [32, H*RCH*W] per chunk, head h at
            # cols h*RCH*W, row r_loc at + r_loc*W.
            CHW = RCH * W
            qsh = ctx.enter_context(tc.tile_pool(name="qsh", bufs=2))

            # ---------------- phase A: LN + transpose + q/k/v projections
            with tc.tile_pool(name="abp", bufs=1) as abp, \
                 tc.tile_pool(name="lnw", bufs=4) as lnw, \
                 tc.tile_pool(name="tpp", bufs=2, space="PSUM") as tpp, \
                 tc.tile_pool(name="pjp", bufs=3, space="PSUM") as pjp:
                xnT = [abp.tile([P, TOK], BF16, tag=f"xnT{kt}",
                                name=f"xnT{kt}") for kt in range(2)]

                def proj_qkg(ws, b, ch, pp):
                    for kt in range(2):
                        nc.tensor.matmul(
                            pp[:], ws[kt][:, b * P:(b + 1) * P],
                            xnT[kt][:, ch * 512:(ch + 1) * 512],
                            start=(kt == 0), stop=(kt == 1))

                drain_rr = [0]

                def drain(dst, src):
                    # gpsimd cannot access PSUM: scalar/vector only
                    e = drain_rr[0] % 2
                    drain_rr[0] += 1
                    if e == 0:
                        nc.scalar.copy(dst, src)
                    else:
                        nc.vector.tensor_copy(out=dst, in_=src)

                xall = abp.tile([P, NT, DN], BF16, tag="xall",
                                name="xall")
                # x chunk 0 first: LN stats for group 0 gate everything
                nc.sync.dma_start(out=xall[:, 0:8, :], in_=x_d[:, 0:8, :])
                nc.sync.dma_start(out=wb, in_=wb_d)
                nc.sync.dma_start(out=xall[:, 8:NT, :],
                                  in_=x_d[:, 8:NT, :])
                nc.scalar.dma_start(out=ebt_sb, in_=ebt_d)
                for g in range(NT // 4):
                    xb = xall[:, 4 * g:4 * g + 4, :]
                    # batched LN stats: 2x bn_stats over 2 chunks each,
                    # then tiny [128, 4] scalar/DVE ops.
                    st = lnw.tile([P, 4, 6], F32, tag="st")
                    for c in range(4):
                        nc.vector.bn_stats(out=st[:, c, :], in_=xb[:, c, :])
                    mv = lnw.tile([P, 4, 2], F32, tag="mv")
                    for c in range(4):
                        nc.vector.bn_aggr(out=mv[:, c, :], in_=st[:, c, :])
                    sd = lnw.tile([P, 4], F32, tag="sd")
                    nc.scalar.activation(sd, mv[:, :, 1], AF.Sqrt,
                                         bias=eps_sb[:])
                    rstd = lnw.tile([P, 4], F32, tag="rs")
                    nc.vector.reciprocal(rstd, sd)
                    nmr = lnw.tile([P, 4], F32, tag="nm")
                    nc.vector.scalar_tensor_tensor(
                        out=nmr, in0=mv[:, :, 0], scalar=-1.0, in1=rstd,
                        op0=MUL, op1=MUL)
                    xng = lnw.tile([P, 4 * DN], BF16, tag="xn")
                    tp_ps = tpp.tile([P, 1024], BF16, tag="tp",
                                     name=f"tp{g}")
                    for c in range(4):
                        xn = xng[:, c * DN:(c + 1) * DN]
                        if c % 2 == 0:
                            nc.scalar.activation(
                                xn, xb[:, c, :], AF.Identity,
                                bias=nmr[:, c:c + 1], scale=rstd[:, c:c + 1])
                        else:
                            nc.vector.tensor_scalar(
                                out=xn, in0=xb[:, c, :],
                                scalar1=rstd[:, c:c + 1],
                                scalar2=nmr[:, c:c + 1],
                                op0=MUL, op1=ADD)
                        for kt in range(2):
                            nc.tensor.transpose(
                                tp_ps[:, kt * 512 + c * P:
                                      kt * 512 + (c + 1) * P],
                                xn[:, kt * P:(kt + 1) * P], ident[:])
                    for kt in range(2):
                        drain(xnT[kt][:, g * 512:(g + 1) * 512],
                              tp_ps[:, kt * 512:(kt + 1) * 512])

                    # q/k/v projections for this chunk
                    ch = g
                    for b in range(NB):
                        pp = pjp.tile([P, 512], F32, tag="pj")
                        proj_qkg(wq, b, ch, pp)
                        drain(qk_sb[:, b, ch * 512:(ch + 1) * 512], pp)
                        pp = pjp.tile([P, 512], F32, tag="pj")
                        proj_qkg(wk, b, ch, pp)
                        drain(qk_sb[:, 2 + b, ch * 512:(ch + 1) * 512], pp)
                    for tl in range(4):
                        tt = g * 4 + tl
                        pp = pjp.tile([P, SLOTS], F32, tag="vps", bufs=2)
                        for kt in range(2):
                            nc.tensor.matmul(
                                pp[:], xnT[kt][:, tt * P:(tt + 1) * P],
                                wv[kt][:], start=(kt == 0), stop=(kt == 1))
                        drain(v_sb[tt][:], pp)

                    for rloc in range(2):
                        r_ = 2 * g + rloc
                        vb_ps = pjp.tile([P, 2], F32, tag="vb", bufs=1)
                        for b in range(NB):
                            for tl in range(2):
                                nc.tensor.matmul(
                                    vb_ps[:, b:b + 1],
                                    v_sb[2 * r_ + tl][:, b * P:(b + 1) * P],
                                    onecol, start=(tl == 0),
                                    stop=(tl == 1))
                        nc.vector.tensor_scalar(
                            out=vbarW_all[:, 2 * r_:2 * r_ + 2], in0=vb_ps,
                            scalar1=1.0 / W, scalar2=None, op0=MUL)

                # g projection + sigmoid (sigmoid act table)
                for ch in range(TOK // 512):
                    for b in range(NB):
                        pp = pjp.tile([P, 512], F32, tag="pj")
                        proj_qkg(wg, b, ch, pp)
                        nc.scalar.activation(
                            sig_sb[b][:, ch * 512:(ch + 1) * 512], pp,
                            AF.Sigmoid, bias=bg[:, b:b + 1])

            # ---------------- phase C: per-row attention (exp table)
            with tc.tile_pool(name="etp", bufs=4) as etp, \
                 tc.tile_pool(name="emp", bufs=4) as emp, \
                 tc.tile_pool(name="rowp", bufs=3) as rowp, \
                 tc.tile_pool(name="ogp", bufs=4) as ogp, \
                 tc.tile_pool(name="dtp", bufs=2, space="PSUM") as dtp, \
                 tc.tile_pool(name="adp", bufs=2, space="PSUM") as adp, \
                 tc.tile_pool(name="opp", bufs=1, space="PSUM") as opp:
                def load_qk_chunk(c):
                    qkt = qsh.tile([32, 4, 4, CHW], BF16, tag="qsh",
                                   name=f"qkc{c}")
                    for u in range(4):
                        (nc.sync if u % 2 == 0 else nc.scalar).dma_start(
                            out=qkt[:, u, :, :],
                            in_=qk_sb[32 * u:32 * u + 32, :,
                                      c * CHW:(c + 1) * CHW])
                    return qkt

                def outproj(pr, pogrs):
                    pt0 = pr * W
                    ot2 = rowp.tile([P, 2 * DN], BF16, tag="ot2")
                    for it in range(2):
                        op_ps = opp.tile([P, DN], F32, tag="op")
                        nc.tensor.matmul(op_ps[:], one1, bo[:],
                                         start=True, stop=False)
                        for b in range(NB):
                            nc.tensor.matmul(
                                op_ps[:], pogrs[b][:, it * P:(it + 1) * P],
                                wo[b][:], start=False, stop=(b == NB - 1))
                        if it % 2 == 0:
                            nc.scalar.copy(ot2[:, 0:DN], op_ps)
                        else:
                            nc.vector.tensor_copy(out=ot2[:, DN:2 * DN],
                                                  in_=op_ps)
                    (nc.sync if pr % 2 == 0 else nc.scalar).dma_start(
                        out=bass.AP(tensor=o_d.tensor, offset=pt0 * DN,
                                    ap=[[DN, P], [P * DN, 2], [1, DN]]),
                        in_=ot2)

                prev = None
                chunks = [load_qk_chunk(0), load_qk_chunk(1)]
                for r in range(DO_C_ROWS):
                    t0 = r * W
                    c, rl = r // RCH, r % RCH
                    qkt = chunks[c % 2]
                    if rl == 0 and c + 1 < NCH and r > 0:
                        chunks[(c + 1) % 2] = load_qk_chunk(c + 1)

                    # 1) dots (untiled, partitions 0:32) + exp + ebias
                    # multiply per (b, jt) group.
                    ems = [[None, None] for _ in range(NB)]
                    for b in range(NB):
                        for jt in range(2):
                            dt_ps = dtp.tile([P, 1024], F32, tag="dt")
                            for u in range(4):
                                hb = rl * W
                                nc.tensor.matmul(
                                    dt_ps[:, u * W:(u + 1) * W],
                                    qkt[:, u, 2 + b,
                                        hb + jt * P:hb + (jt + 1) * P],
                                    qkt[:, u, b, hb:hb + W],
                                    start=True, stop=True)
                            et = etp.tile([P, 1024], BF16, tag="et")
                            nc.scalar.activation(
                                et, dt_ps, AF.Exp,
                                bias=ngj[:, r * 2 + jt:r * 2 + jt + 1])
                            em = emp.tile([P, 1024], BF16, tag="em")
                            eo = (2 * b + jt) * 1024
                            if jt == 0:
                                nc.vector.tensor_tensor(
                                    out=em, in0=et,
                                    in1=ebt_sb[:, eo:eo + 1024], op=MUL)
                            else:
                                # gpsimd is ~3x slower per element: give it
                                # only the last quarter
                                nc.vector.tensor_tensor(
                                    out=em[:, 0:768], in0=et[:, 0:768],
                                    in1=ebt_sb[:, eo:eo + 768], op=MUL)
                                nc.gpsimd.tensor_tensor(
                                    out=em[:, 768:1024],
                                    in0=et[:, 768:1024],
                                    in1=ebt_sb[:, eo + 768:eo + 1024],
                                    op=MUL)
                            ems[b][jt] = em

                    # software pipeline: the previous row's output
                    # projection lands here on the PE queue.
                    if prev is not None:
                        outproj(*prev)
                        prev = None

                    # 2) attn@v + denominators (col-tiled bands; each
                    # accumulation group start->stop contiguous).
                    avdns = []
                    for b in range(NB):
                        avdn = adp.tile([P, 512], F32, tag="ad",
                                        name=f"ad{r}_{b}")
                        ems_b = ems[b]
                        for u in range(4):
                            ho = 32 * u
                            for jt in range(2):
                                nc.tensor.matmul(
                                    avdn[ho:ho + 32, 0:W],
                                    v_sb[2 * r + jt][:, b * P + ho:
                                                     b * P + ho + DH],
                                    ems_b[jt][:, u * W:(u + 1) * W],
                                    start=(jt == 0), stop=(jt == 1),
                                    tile_position=(0, ho))
                            for jt in range(2):
                                nc.tensor.matmul(
                                    avdn[ho:ho + 32, W:2 * W],
                                    ones_blk,
                                    ems_b[jt][:, u * W:(u + 1) * W],
                                    start=(jt == 0), stop=(jt == 1),
                                    tile_position=(0, ho))
                        avdns.append(avdn)

                    # 3) normalize + gate + masked-row fixup:
                    #    ogr = ((av*rbig - vbar)*m + vbar) * sig
                    ogrs = []
                    for b in range(NB):
                        avdn = avdns[b]
                        vcol = vbarW_all[:, 2 * r + b:2 * r + b + 1]
                        rbig = rowp.tile([P, W], F32, tag="rbig")
                        nc.vector.reciprocal_approx_fast(rbig, avdn[:, W:])
                        t1 = rowp.tile([P, W], F32, tag="t1")
                        nc.vector.tensor_tensor(
                            out=t1, in0=avdn[:, 0:W], in1=rbig, op=MUL)
                        d = rowp.tile([P, W], F32, tag="d")
                        nc.vector.scalar_tensor_tensor(
                            out=d, in0=t1, scalar=vcol,
                            in1=mf_all[:, t0:t0 + W], op0=SUB, op1=MUL)
                        ogr = ogp.tile([P, W], BF16, tag=f"og{b}")
                        nc.vector.scalar_tensor_tensor(
                            out=ogr, in0=d, scalar=vcol,
                            in1=sig_sb[b][:, t0:t0 + W], op0=ADD, op1=MUL)
                        ogrs.append(ogr)

                    prev = (r, ogrs)
                if prev is not None:
                    outproj(*prev)
    nc.compile()
    return nc


_NC_CACHE = {}
TRACE = False
SIM_TRACE = False
DO_C_ROWS = RPC
DBG = False


def _get_nc(name):
    if name not in _NC_CACHE:
        _NC_CACHE[name] = (_build_bias_nc if name == "bias"
                           else _build_attn_nc)()
    return _NC_CACHE[name]


def _prep(x, edges, mask, edge_mask, ln_g, ln_b, lne_g, lne_b,
          W_edge, Wq, Wkv, Wg, bg, Wo, bo):
    f32 = np.float32
    bf16 = ml_dtypes.bfloat16
    x = np.asarray(x, f32)
    edges = np.asarray(edges, f32)
    mask_b = np.asarray(mask).astype(bool)
    edge_mask_b = np.asarray(edge_mask).astype(bool)
    ln_g = np.asarray(ln_g, f32); ln_b = np.asarray(ln_b, f32)
    lne_g = np.asarray(lne_g, f32); lne_b = np.asarray(lne_b, f32)
    W_edge = np.asarray(W_edge, f32)
    Wq = np.asarray(Wq, f32); Wkv = np.asarray(Wkv, f32)
    Wg = np.asarray(Wg, f32); bg = np.asarray(bg, f32)
    Wo = np.asarray(Wo, f32); bo = np.asarray(bo, f32)

    # ---------------- kernel 1: raw edge products
    nc1 = _get_nc("bias")
    we = (lne_g[:, None] * W_edge).astype(f32)
    we_bf = we.astype(bf16)
    wa = np.zeros((DE, 9), f32)
    wa[:, 0:8] = we_bf.astype(f32)
    wa[:, 8] = 1.0 / DE
    e_flat = edges.reshape(W, W, DE)
    in_maps1 = []
    for c in range(NC):
        in_maps1.append({
            "e": np.ascontiguousarray(
                e_flat[c * IPC:(c + 1) * IPC].reshape(IPC * W, DE).T
            ).astype(bf16),
            "wa": wa.astype(bf16),
        })
    res1 = _run_spmd(nc1, in_maps1)
    if TRACE:
        print("bias kernel exec_time_ns:", res1.exec_time_ns)
    o1 = np.concatenate([res1.results[c]["o"] for c in range(NC)],
                        axis=1)  # [10, W*W]
    raw = o1[0:8]                        # [8, i*j]
    mu = o1[8]                           # [i*j]
    var = o1[9] / DE - mu * mu
    rstd = 1.0 / np.sqrt(var + EPS)
    swe = we_bf.astype(f32).sum(axis=0)  # [H]
    bias = rstd[None, :] * (raw - mu[None, :] * swe[:, None])
    bias = bias.reshape(H, W, W) + (lne_b @ W_edge)[:, None, None]
    # exp(bias) with masked edges -> exactly 0 (exp identity moves the
    # bias out of the PE: exp(qk+bias) = exp(qk) * exp(bias))
    ebias = np.where(edge_mask_b[0][None], np.exp(bias), 0.0)  # [H, i, j]
    ebT = ebias.transpose(0, 2, 1)                             # [H, j, i]
    ebt = np.ascontiguousarray(
        ebT.reshape(NB, 4, 2, P, W)            # (b, u, jt, p, i)
        .transpose(3, 0, 2, 1, 4)              # (p, b, jt, u, i)
        .reshape(P, NB * 2 * 1024)).astype(f32)

    # ---------------- kernel 2: attention
    nc2 = _get_nc("attn")
    scale = DH ** -0.5
    Wk_, Wv_ = Wkv[:, :H * DH], Wkv[:, H * DH:]
    gq = _expand_cols((ln_g[:, None] * Wq * scale).astype(f32))
    gk = _expand_cols((ln_g[:, None] * Wk_).astype(f32))
    gv = _expand_cols((ln_g[:, None] * Wv_).astype(f32))
    gg = _expand_cols((ln_g[:, None] * Wg).astype(f32))
    assert np.allclose(ln_b, 0.0), "ln_b folding not implemented"
    bgx = np.zeros((P, NB), f32)
    for h in range(H):
        bgx[32 * (h % 4):32 * (h % 4) + DH, h // 4] = \
            bg[h * DH:(h + 1) * DH]
    woe = _expand_rows(Wo.astype(f32))

    maskf = mask_b[0].astype(f32)  # [R, W]
    x_flat = x.reshape(R, W, DN)
    in_maps2 = []
    for c in range(NC):
        mrows = maskf[c * RPC:(c + 1) * RPC]  # [RPC, W]
        ngj = (mrows.reshape(RPC, 2, P) - 1.0) * 1e38  # [r, jt, p]
        ngj = np.ascontiguousarray(
            ngj.transpose(2, 0, 1).reshape(P, RPC * 2))
        # bf16 constant blob: wq|wk|wv|wg|wo|bo(row0)|idm|onesb|mf|ebt
        wbb = np.zeros((P, WB_COLS), f32)
        wbb[:, 0:512] = gq.reshape(2, P, SLOTS).transpose(1, 0, 2) \
            .reshape(P, 512)
        wbb[:, 512:1024] = gk.reshape(2, P, SLOTS) \
            .transpose(1, 0, 2).reshape(P, 512)
        wbb[:, 1024:1536] = gv.reshape(2, P, SLOTS) \
            .transpose(1, 0, 2).reshape(P, 512)
        wbb[:, 1536:2048] = gg.reshape(2, P, SLOTS) \
            .transpose(1, 0, 2).reshape(P, 512)
        wbb[:, 2048:2560] = woe.reshape(2, P, DN).transpose(1, 0, 2) \
            .reshape(P, 512)
        wbb[0, 2560:2816] = bo
        wbb[:, 2816:2944] = np.eye(P, dtype=f32)
        wbb[:, 2944:3072] = 1.0
        fbb = np.concatenate([bgx, ngj], axis=1)
        xc = x_flat[c * RPC:(c + 1) * RPC].reshape(NT, P, DN)
        in_maps2.append({
            "x": np.ascontiguousarray(xc.transpose(1, 0, 2)).astype(bf16),
            "wb": wbb.astype(bf16),
            "fb": fbb.astype(f32),
            "ebt": ebt.astype(bf16),
            "mf": mrows.astype(bf16),
        })
    return nc2, in_maps2


def _run_spmd(nc, in_maps):
    # the axon-tunneled devices occasionally fail one execution with
    # NRT_EXEC_UNIT_UNRECOVERABLE; a retry succeeds
    try:
        return bass_utils.run_bass_kernel_spmd(nc, in_maps,
                                               core_ids=list(range(NC)),
                                               trace=TRACE)
    except Exception:
        return bass_utils.run_bass_kernel_spmd(nc, in_maps,
                                               core_ids=list(range(NC)),
                                               trace=TRACE)


def kernel(**inputs):
    nc2, in_maps2 = _prep(**inputs)
    res2 = _run_spmd(nc2, in_maps2)
    if TRACE:
        print("attn kernel exec_time_ns:", res2.exec_time_ns)
    out = np.concatenate(
        [res2.results[c]["o"].astype(np.float32).reshape(RPC, W, DN)
         for c in range(NC)],
        axis=0)
    return out.reshape(B, R, W, DN).astype(np.float32)
